# revision 18
# baseline (speedup 1.0000x reference)
"""BC6H surrogate block-level decode kernel for 8 Trainium2 NeuronCores.

Full-input contract: kernel(**inputs) takes the complete arrays from
setup_inputs() and returns the full (3, 4096, 4096) image.  Internally the
block dimension (nb = 1048576) is sharded 8 ways (pure data parallel); each
core runs an identical Bass/Tile program on its 131072-block shard.

Math (per 4x4 block b, pixel p in 0..15, channel c in 0..2):
  sig_e = sigmoid(endpoints)                      (4 endpoints x 3 ch)
  w     = (63*sig(idx) + clip(7*sig(idx)-3,0,1))/64      exact LUT lerp
  m     = softmax(logits) @ bank                  (soft partition mask)
  e_u_i = 31248*sig_e_i + 248                     (uf16-domain endpoints)
  y     = m*(e0(1-w)+e1 w) + (1-m)*(e2(1-w)+e3 w)
  u     = y/1024 ;  hh = clip(floor(u - 1/1024) - 1, 0, 31)
  out   = 2^(hh-14) * (u - hh)

Implementation notes:
  * block-major layout: SBUF tile row r holds blocks b0 + r*G + g, so every
    DRAM transfer is fully contiguous per partition.
  * softmax@bank runs on the TensorEngine: PE-transpose raw logits
    (128x128 chunks) into PSUM, ACT exp's them back to SBUF transposed,
    then per 128-block group one matmul E_T.T @ [bank3 | ones] yields
    num[b, (c,p)] (bank replicated over c) and den[b] in PSUM, block-major.
    u never needs an explicit softmax: u = R + (num * (1/den)-folded coeffs).
  * the weight-LUT lerp has an exact closed form (one custom DVE op).
  * floor() is the fp32 magic-number round trick:
    (relu(v') - 0.5 + 1.5*2^23) - 1.5*2^23 = floor(v') for our value range
    (v' < 31 always, so the hh<=31 clip is dead; boundary ties are benign).
  * 2^(hh-14) = ACT Exp(ln2*hh - 14*ln2), exact to ~2 ULP for integer hh.
"""

import sys

sys.path.insert(0, "/opt/trn_rl_repo")

from contextlib import ExitStack

import numpy as np

import concourse.bass as bass
import concourse.tile as tile
from concourse import bacc, mybir
from concourse import bass_utils
from concourse import dve_ops
from concourse.dve_ops import DveOp
from concourse.dve_spec import (
    Spec,
    Src0,
    C0,
    C1,
    C2,
    One,
    relu,
    minn,
    lower,
    _has_src1,
)
from concourse.dve_uop import DveOpSpec

F32 = mybir.dt.float32
AOp = mybir.AluOpType

# ---------------------------------------------------------------- constants
NB = 1048576
N_CORES = 8
NB_CORE = NB // N_CORES            # 131072 blocks per core
G = 32                             # blocks per partition-row per supertile
H = W = 4096
BY = BX = 1024

# e_u = 31248*sig + 248 ; u = y/1024 -> u = 30.515625*sig_combo + 0.2421875
EU_SCALE = 31248.0 / 1024.0        # 30.515625
EU_BIAS = 248.0 / 1024.0           # 0.2421875
FLOOR_OFF = 1.0 + 1.0 / 1024.0     # v' = u - FLOOR_OFF ; hh = relu-floor(v')
# floor(v') = round(relu(v' - 0.5)) via the f32 magic-add trick.  The -0.5
# must fold into the relu threshold: MAGIC - 0.5 is not representable in f32.
FLOOR_OFF_H = FLOOR_OFF + 0.5      # 1.5009765625 (exact in f32)
MAGIC = 12582912.0                 # 1.5 * 2^23
LN2 = 0.6931471805599453

# ------------------------------------------------------- custom DVE ops
_REGISTERED = {}


def _register(name, spec):
    if name in _REGISTERED:
        return _REGISTERED[name]
    if name not in dve_ops._SUB_OPCODE_FOR_NAME:
        row = max(dve_ops._SUB_OPCODE_FOR_NAME.values()) + 1
        assert row < 0x20, "custom-DVE opcode rows exhausted"
        dve_ops._SUB_OPCODE_FOR_NAME[name] = row
    row = dve_ops._SUB_OPCODE_FOR_NAME[name]
    shas = {}
    for ver in ("v3", "v4"):
        try:
            uops = lower(spec, ver=ver)
            shas[ver] = DveOpSpec(
                name=name, opcode=row, uops=uops, rd1_en=_has_src1(spec)
            ).sha(ver)
        except Exception:
            if ver == "v3":
                raise
    op = DveOp(name, spec, subdim=False, uops_sha=shas)
    dve_ops.OPS.append(op)
    dve_ops.CUSTOM_DVE_SPECS[name] = op.spec
    _REGISTERED[name] = op
    return op


# w = s - (s - min(relu(s*c0 + c1), 1))*c2 ; c0=7, c1=-3, c2=1/64
#   = (63 s + clip(7s-3, 0, 1)) / 64
BC6W = _register(
    "BC6W_ANT",
    Spec(
        body=Src0 - (Src0 - minn(relu(Src0 * C0 + C1), One)) * C2,
        reference=lambda in0, in1, c0, c1, c2: (
            in0.astype(np.float32)
            - (
                in0.astype(np.float32)
                - np.minimum(
                    np.maximum(in0.astype(np.float32) * c0 + c1, 0.0), 1.0
                )
            )
            * c2
        ).astype(np.float32),
    ),
)


def _ref_hh(in0, in1, c0, c1, c2):
    x = np.maximum(
        (in0.astype(np.float32) - np.float32(c0)).astype(np.float32), 0.0
    ).astype(np.float32)
    return ((x + np.float32(c1)).astype(np.float32) - np.float32(c2)).astype(
        np.float32
    )


# hh = (relu(u - c0) + c1) - c2 ; c1 = MAGIC-0.5, c2 = MAGIC  -> floor()
BC6HH = _register(
    "BC6HH_ANT",
    Spec(body=(relu(Src0 - C0) + C1) - C2, reference=_ref_hh),
)

# frac = u - hh(u)  (recomputes hh inline; 1 stream)
BC6FRAC = _register(
    "BC6FRAC_ANT",
    Spec(
        body=Src0 - ((relu(Src0 - C0) + C1) - C2),
        reference=lambda in0, in1, c0, c1, c2: (
            in0.astype(np.float32) - _ref_hh(in0, in1, c0, c1, c2)
        ).astype(np.float32),
    ),
)


# ------------------------------------------------------- bass kernel build
def _ap4(base, dims):
    """Manual free-dim AP: keep base's partition dim, set free dims."""
    return bass.AP(base.tensor, base.offset, [list(base.ap[0])] + dims)


def build_kernel(nb_core=NB_CORE, g=G, dbg=None, stop="all"):
    st_blocks = 128 * g
    n_st = nb_core // st_blocks
    assert nb_core % st_blocks == 0
    assert g % 4 == 0

    nc = bacc.Bacc(
        "TRN2",
        target_bir_lowering=False,
        debug=False,
        enable_asserts=False,
        num_devices=1,
    )

    ep = nc.dram_tensor("endpoints", [nb_core, 12], F32, kind="ExternalInput").ap()
    ix = nc.dram_tensor("indices", [nb_core, 16], F32, kind="ExternalInput").ap()
    lg = nc.dram_tensor("logits", [nb_core, 32], F32, kind="ExternalInput").ap()
    # bank_diag: [128, 4*49] block-diagonal: row k (band q = k//32) has
    # [bank3[k%32] | 1] in cols 49q..49q+48, zeros elsewhere.  One K=128
    # matmul then computes num/den for 4 groups at once (the 4 partition
    # bands of one transposed chunk), with no PE tile_position switching
    # (mixing tile positions between matmuls faults at runtime).
    bank3 = nc.dram_tensor("bank3", [128, 196], F32, kind="ExternalInput").ap()
    ident = nc.dram_tensor("ident", [128, 128], F32, kind="ExternalInput").ap()
    out = nc.dram_tensor("out", [nb_core, 48], F32, kind="ExternalOutput").ap()

    with tile.TileContext(nc) as tc, ExitStack() as ctx:
        const_pool = ctx.enter_context(tc.tile_pool(name="const", bufs=1))
        in_pool = ctx.enter_context(tc.tile_pool(name="inp", bufs=2))
        mid_pool = ctx.enter_context(tc.tile_pool(name="mid", bufs=2))
        big1_pool = ctx.enter_context(tc.tile_pool(name="big1", bufs=1))
        big2_pool = ctx.enter_context(tc.tile_pool(name="big2", bufs=2))
        out_pool = ctx.enter_context(tc.tile_pool(name="outp", bufs=2))
        ps_t = ctx.enter_context(tc.tile_pool(name="ps_t", bufs=2, space="PSUM"))
        ps_mm = ctx.enter_context(tc.tile_pool(name="ps_mm", bufs=4, space="PSUM"))

        bank_t = const_pool.tile([128, 196], F32)
        nc.sync.dma_start(bank_t[:], bank3)
        id_t = const_pool.tile([128, 128], F32)
        nc.sync.dma_start(id_t[:], ident)

        for t in range(n_st):
            b0 = t * st_blocks
            # ---- loads (contiguous per partition) ----
            ep_t = in_pool.tile([128, g * 12], F32, tag="ep")
            nc.sync.dma_start(
                ep_t[:],
                ep[b0 : b0 + st_blocks, :].rearrange("(r g) d -> r (g d)", g=g),
            )
            ix_t = in_pool.tile([128, g * 16], F32, tag="ix")
            nc.sync.dma_start(
                ix_t[:],
                ix[b0 : b0 + st_blocks, :].rearrange("(r g) d -> r (g d)", g=g),
            )
            lg_t = in_pool.tile([128, g * 32], F32, tag="lg")
            nc.sync.dma_start(
                lg_t[:],
                lg[b0 : b0 + st_blocks, :].rearrange("(r g) d -> r (g d)", g=g),
            )

            # ---- ACT sigmoids (block-major, full partitions) ----
            ep_s = mid_pool.tile([128, g * 12], F32, tag="eps")
            nc.scalar.activation(
                ep_s[:], ep_t[:], mybir.ActivationFunctionType.Sigmoid
            )
            ix_s = mid_pool.tile([128, g * 16], F32, tag="ixs")
            nc.scalar.activation(
                ix_s[:], ix_t[:], mybir.ActivationFunctionType.Sigmoid
            )

            # ---- w (custom DVE, one pass) ----
            w_t = mid_pool.tile([128, g * 16], F32, tag="w")
            nc.vector._custom_dve(
                BC6W, out=w_t[:], in0=ix_s[:], s0=7.0, s1=-3.0, imm2=1.0 / 64.0
            )

            if stop == "sig":
                o_t = out_pool.tile([128, g * 48], F32, tag="o")
                nc.vector.tensor_copy(o_t[:, 0 : g * 16], w_t[:])
                nc.vector.tensor_copy(o_t[:, g * 16 : g * 28], ep_s[:])
                nc.vector.tensor_copy(o_t[:, g * 28 : g * 44], ix_s[:])
                nc.vector.tensor_copy(
                    o_t[:, g * 44 : g * 48], ix_s[:, 0 : g * 4]
                )
                nc.sync.dma_start(
                    out[b0 : b0 + st_blocks, :].rearrange(
                        "(r g) d -> r (g d)", g=g
                    ),
                    o_t[:],
                )
                continue
            # ---- endpoint combos (small strided ops) ----
            ep3 = ep_s[:, :].rearrange("r (g d) -> r g d", g=g)

            def eslice(i):  # sigmoid of endpoint i: [128, g, 3]
                return ep3[:, :, 3 * i : 3 * i + 3]

            s2u = mid_pool.tile([128, g * 3], F32, tag="s2u")
            s2u3 = s2u[:, :].rearrange("r (g c) -> r g c", g=g)
            nc.vector.tensor_scalar(
                s2u3, eslice(2), EU_SCALE, EU_BIAS, AOp.mult, AOp.add
            )
            d32 = mid_pool.tile([128, g * 3], F32, tag="d32")  # sig3-sig2
            d32v = d32[:, :].rearrange("r (g c) -> r g c", g=g)
            nc.vector.tensor_sub(d32v, eslice(3), eslice(2))
            bu = mid_pool.tile([128, g * 3], F32, tag="bu")  # Bu = EU_SCALE*d32
            bu3 = bu[:, :].rearrange("r (g c) -> r g c", g=g)
            nc.vector.tensor_scalar_mul(bu3, d32v, EU_SCALE)
            d02 = mid_pool.tile([128, g * 3], F32, tag="d02")  # sig0-sig2
            d02v = d02[:, :].rearrange("r (g c) -> r g c", g=g)
            nc.vector.tensor_sub(d02v, eslice(0), eslice(2))
            d13 = mid_pool.tile([128, g * 3], F32, tag="d13")  # sig1-sig3
            d13v = d13[:, :].rearrange("r (g c) -> r g c", g=g)
            nc.vector.tensor_sub(d13v, eslice(1), eslice(3))
            dd = mid_pool.tile([128, g * 3], F32, tag="dd")  # D/EU = d13-d02
            ddv = dd[:, :].rearrange("r (g c) -> r g c", g=g)
            nc.vector.tensor_sub(ddv, d13v, d02v)

            # ---- logits: PE transpose -> ACT exp -> E_T in SBUF ----
            n_ch = g // 4  # chunks of 4 groups (512 blocks)
            e_T = big2_pool.tile([128, g * 32], F32, tag="eT")
            for j in range(0, n_ch, 4):
                jn = min(4, n_ch - j)
                pst = ps_t.tile([128, 512], F32, tag="pst")
                for q in range(jn):
                    ch = j + q
                    nc.tensor.transpose(
                        pst[:, 128 * q : 128 * (q + 1)],
                        lg_t[:, 128 * ch : 128 * (ch + 1)],
                        id_t[:],
                    )
                nc.scalar.activation(
                    e_T[:, 128 * j : 128 * (j + jn)],
                    pst[:, : 128 * jn],
                    mybir.ActivationFunctionType.Exp,
                )

            if stop == "eT":
                o_t = out_pool.tile([128, g * 48], F32, tag="o")
                nc.vector.tensor_copy(o_t[:, 0 : g * 32], e_T[:])
                nc.vector.tensor_copy(
                    o_t[:, g * 32 : g * 48], w_t[:]
                )
                nc.sync.dma_start(
                    out[b0 : b0 + st_blocks, :].rearrange(
                        "(r g) d -> r (g d)", g=g
                    ),
                    o_t[:],
                )
                continue
            # ---- per-chunk matmuls: [num | den] x4 groups into PSUM ----
            # two chunks share one PSUM bank (2*196 f32 <= 512) so only
            # n_ch/2 PSUM tiles stay live until the z-phase consumes them.
            rcp = mid_pool.tile([128, g], F32, tag="rcp")
            num_tiles = []
            pmm = None
            for ch in range(n_ch):
                off = 196 * (ch % 2)
                if off == 0:
                    pmm = ps_mm.tile([128, 392], F32, tag="pmm")
                nc.tensor.matmul(
                    pmm[:, off : off + 196],
                    e_T[:, 128 * ch : 128 * (ch + 1)],
                    bank_t[:, :],
                    start=True,
                    stop=True,
                )
                nc.vector.reciprocal(
                    rcp[:, 4 * ch : 4 * ch + 4],
                    _ap4(pmm[:, off + 48 :], [[49, 4]]),
                )
                num_tiles.append((ch, pmm, off))

            if stop in ("mask", "mask1band", "maskb32", "maskb64", "maskb96", "maskb03"):
                o_t = out_pool.tile([128, g * 48], F32, tag="o")
                for (ch, pmm, off) in num_tiles:
                    o_s = _ap4(
                        o_t[:, 48 * 4 * ch : 48 * 4 * (ch + 1)],
                        [[48, 4], [1, 48]],
                    )
                    nc.vector.tensor_copy(
                        o_s, _ap4(pmm[:, off : off + 196], [[49, 4], [1, 48]])
                    )
                nc.sync.dma_start(
                    out[b0 : b0 + st_blocks, :].rearrange(
                        "(r g) d -> r (g d)", g=g
                    ),
                    o_t[:],
                )
                continue
            # ---- fold 1/den into C, D coefficients ----
            rcp_b = rcp[:, :].broadcast_to([128, g, 3])
            cur = mid_pool.tile([128, g * 3], F32, tag="cur")
            cur3 = cur[:, :].rearrange("r (g c) -> r g c", g=g)
            nc.vector.tensor_mul(cur3, d02v, rcp_b)
            nc.vector.tensor_scalar_mul(cur3, cur3, EU_SCALE)
            dur = mid_pool.tile([128, g * 3], F32, tag="dur")
            dur3 = dur[:, :].rearrange("r (g c) -> r g c", g=g)
            nc.vector.tensor_mul(dur3, ddv, rcp_b)
            nc.vector.tensor_scalar_mul(dur3, dur3, EU_SCALE)

            # ---- z assembly: u = S2u + Bu*w + (Cur + Dur*w) * num ----
            w_b = _ap4(w_t[:, :], [[16, g], [0, 3], [1, 16]])

            def cb(tile_):  # [128, g*3] -> [r, g, c, p] broadcast over p
                return tile_[:, :].rearrange("r (g c) -> r g c", g=g).broadcast_to(
                    [128, g, 3, 16]
                )

            tA = big1_pool.tile([128, g * 48], F32, tag="tA")
            tA4 = tA[:, :].rearrange("r (g c p) -> r g c p", g=g, c=3)
            tB = big1_pool.tile([128, g * 48], F32, tag="tB")
            tB4 = tB[:, :].rearrange("r (g c p) -> r g c p", g=g, c=3)
            u_t = big2_pool.tile([128, g * 48], F32, tag="u")
            u4 = u_t[:, :].rearrange("r (g c p) -> r g c p", g=g, c=3)

            nc.vector.tensor_mul(tA4, cb(dur), w_b)            # Dur*w
            nc.vector.tensor_add(tA4, tA4, cb(cur))            # + Cur
            for (ch, pmm, off) in num_tiles:                   # * num (PSUM)
                # PSUM operands only support 2 free dims: read num as
                # [[49, 4], [1, 48]] (the (c,p) block is contiguous).
                num_b = _ap4(pmm[:, off : off + 196], [[49, 4], [1, 48]])
                tA_s = _ap4(
                    tA[:, 48 * 4 * ch : 48 * 4 * (ch + 1)], [[48, 4], [1, 48]]
                )
                nc.vector.tensor_mul(tA_s, tA_s, num_b)
            nc.vector.tensor_mul(tB4, cb(bu), w_b)             # Bu*w
            nc.gpsimd.tensor_add(tA4, tA4, tB4)                # + Bu*w
            nc.vector.tensor_add(u4, tA4, cb(s2u))             # + S2u

            if stop == "u":
                o_t = out_pool.tile([128, g * 48], F32, tag="o")
                nc.vector.tensor_copy(o_t[:], u_t[:])
                nc.sync.dma_start(
                    out[b0 : b0 + st_blocks, :].rearrange(
                        "(r g) d -> r (g d)", g=g
                    ),
                    o_t[:],
                )
                continue
            # ---- decode ----
            hh_t = big1_pool.tile([128, g * 48], F32, tag="hh")
            # hh = floor(relu(u - FLOOR_OFF)) via 2 gpsimd tensor_scalar ops
            nc.gpsimd.tensor_scalar(
                hh_t[:], u_t[:], FLOOR_OFF_H, 0.0, AOp.subtract, AOp.max
            )
            # second op also folds the -14 exponent bias: hh_t := hh - 14
            nc.gpsimd.tensor_scalar(
                hh_t[:], hh_t[:], MAGIC, MAGIC + 14.0, AOp.add, AOp.subtract
            )
            e2_t = big1_pool.tile([128, g * 48], F32, tag="e2")
            nc.scalar.activation(
                e2_t[:],
                hh_t[:],
                mybir.ActivationFunctionType.Exp,
                bias=0.0,
                scale=LN2,
            )
            fr_t = big1_pool.tile([128, g * 48], F32, tag="fr")
            nc.vector._custom_dve(
                BC6FRAC,
                out=fr_t[:],
                in0=u_t[:],
                s0=FLOOR_OFF_H,
                s1=MAGIC,
                imm2=MAGIC,
            )
            o_t = out_pool.tile([128, g * 48], F32, tag="o")
            nc.gpsimd.tensor_mul(o_t[:], fr_t[:], e2_t[:])

            if dbg is not None:
                src_map = {"u": u_t, "hh": hh_t, "fr": fr_t, "e2": e2_t,
                           "tA": tA, "tB": tB}
                if dbg == "num":
                    # dump raw num (PSUM) via copy: [128, g*48]
                    for (j0, jn, pmm) in num_tiles:
                        nc.vector.tensor_copy(
                            o_t[:, :].rearrange(
                                "r (g c p) -> r g c p", g=g, c=3
                            )[:, j0 : j0 + jn, :, :],
                            _ap4(pmm[:, :], [[49, jn], [16, 3], [1, 16]]),
                        )
                elif dbg == "rcp":
                    nc.vector.tensor_copy(
                        o_t[:, 0 : g], rcp[:, :]
                    )
                else:
                    nc.vector.tensor_copy(o_t[:], src_map[dbg][:])

            nc.sync.dma_start(
                out[b0 : b0 + st_blocks, :].rearrange("(r g) d -> r (g d)", g=g),
                o_t[:],
            )

    nc.compile()
    return nc


# ------------------------------------------------------- host-side driver
_NC_CACHE = {}


def _get_nc():
    if "nc" not in _NC_CACHE:
        _NC_CACHE["nc"] = build_kernel()
    return _NC_CACHE["nc"]


def make_in_maps(endpoints, indices, partition_logits, partition_bank, nb=NB):
    """Shard + pack host inputs into the 8 per-core input dicts."""
    b49 = np.empty((32, 49), dtype=np.float32)
    b49[:, 0:48] = np.tile(partition_bank.astype(np.float32), (1, 3)).reshape(
        32, 48
    )
    b49[:, 48] = 1.0
    bank3 = np.zeros((128, 196), dtype=np.float32)
    for q in range(4):
        bank3[32 * q : 32 * (q + 1), 49 * q : 49 * (q + 1)] = b49
    ident = np.eye(128, dtype=np.float32)

    ep_flat = np.ascontiguousarray(
        endpoints.astype(np.float32).reshape(nb, 12)
    )
    ixf = np.ascontiguousarray(indices.astype(np.float32))
    lgf = np.ascontiguousarray(partition_logits.astype(np.float32))
    nbc = nb // N_CORES
    in_maps = []
    for c in range(N_CORES):
        sl = slice(c * nbc, (c + 1) * nbc)
        in_maps.append(
            {
                "endpoints": np.ascontiguousarray(ep_flat[sl]),
                "indices": np.ascontiguousarray(ixf[sl]),
                "logits": np.ascontiguousarray(lgf[sl]),
                "bank3": bank3,
                "ident": ident,
            }
        )
    return in_maps


def blocks_to_img(blocks):
    """[NB, 48] c-major blocks -> (3, H, W) image."""
    return (
        blocks.reshape(BY, BX, 3, 4, 4)
        .transpose(2, 0, 3, 1, 4)
        .reshape(3, H, W)
        .astype(np.float32)
    )


def kernel(endpoints, indices, partition_logits, partition_bank, weight_lut):
    endpoints = np.asarray(endpoints, dtype=np.float32)
    indices = np.asarray(indices, dtype=np.float32)
    partition_logits = np.asarray(partition_logits, dtype=np.float32)
    partition_bank = np.asarray(partition_bank, dtype=np.float32)
    assert endpoints.shape[0] == NB

    in_maps = make_in_maps(endpoints, indices, partition_logits, partition_bank)
    nc = _get_nc()
    res = bass_utils.run_bass_kernel_spmd(
        nc, in_maps, core_ids=list(range(N_CORES))
    )
    blocks = np.concatenate(
        [res.results[c]["out"] for c in range(N_CORES)], axis=0
    )
    return blocks_to_img(blocks)
